# revision 34
# baseline (speedup 1.0000x reference)
"""Trainium2 Bass kernel for nn_AcPredict (banded basis-mixture Kalman predict).

Math (validated vs reference in numpy, rel err ~2e-7):
  All four basis stacks are band-masked (|i-j| <= 3), so the per-batch mixed
  transition matrices are 7-diagonal.  With D_m[b,i,t] = sum_k coeff[b,k] *
  basis_m[k,i,i+t-3]  (m in {11,12,21,22} -> 1..4) and S_x[b,i,t] = x[b,i+t-3]:

    nmu = mu + red_t(D1*S_mu + D2*S_ml)
    nml = ml + red_t(D3*S_mu + D4*S_ml)
    P1 = D1*S_cu + D2*S_cs ; P2 = D1*S_cs + D2*S_cl
    P3 = D3*S_cu + D4*S_cs ; P4 = D3*S_cs + D4*S_cl
    ncu = red_t(D1*P1 + D2*P2) + 2*P1[t=3] + cu + pcu
    ncl = red_t(D3*P3 + D4*P4) + 2*P4[t=3] + cl + pcl
    ncs = red_t(D3*P1 + D4*P2) + P2[t=3] + P3[t=3] + cs

  (the diagonal/identity cross terms are exactly the t=3 slices of the P planes)

Sharding: pure data-parallel, batch 4096 -> 8 cores x 512 rows.
S planes are never materialized: reads use a strided AP over a zero-padded
x6p buffer (offset = 70*slot + i + t).

Walrus caps sync-waits per compute instruction at 1, so all inputs are
pre-staged with one DMA per DRAM tensor (4 total), absorbed onto each
engine's vector clock by tiny warm-up ops, and written back with one DMA.
"""

import sys

for _p in ("/opt/trn_rl_repo", "/opt/trn_rl_repo/concourse"):
    if _p not in sys.path:
        sys.path.insert(0, _p)

from contextlib import ExitStack

import ml_dtypes
import numpy as np

import concourse.bass as bass
import concourse.mybir as mybir
from concourse.bass import AP
from concourse.bass_utils import run_bass_kernel_spmd
from concourse.tile import TileContext

F32 = mybir.dt.float32
BF16 = mybir.dt.bfloat16
AX = mybir.AxisListType
OP = mybir.AluOpType
AF = mybir.ActivationFunctionType

B, LOD, LSD, LAD, K, BW, H = 4096, 64, 128, 32, 15, 3, 128
T = 2 * BW + 1          # 7 diagonals
NCORES = 8
R = B // NCORES         # rows per core = 512
P = 128                 # partitions per tile
NT = R // P             # tiles per core = 4
PL = LOD * T            # 448 elements per D plane
SL = LOD + 2 * BW       # 70 = padded slot width in x6p

# bf16 const blob column offsets
CB_E = 0                # e: [15, 1792]
CB_W1 = 4 * PL          # w1t: [128, 128]
CB_W2 = CB_W1 + H       # w2t: [128, 15]
CB_N = CB_W2 + K        # 1935
# f32 const blob column offsets
CF_IDN = 0              # idn: [128, 128]
CF_PCB = P              # pcb: [128, 192]
CF_B1 = CF_PCB + 3 * LOD    # b1: [128, 1]
CF_B2 = CF_B1 + 1           # b2: [15, 1]
CF_N = CF_B2 + 1        # 322


def _sread(x6p_tile, slot0):
    """AP reading S[slot, i, t] = x6p[70*(slot0+slot) + i + t], 3 slots."""
    base = x6p_tile[:, slot0 * SL : slot0 * SL + 1]
    return AP(
        tensor=base.tensor,
        offset=base.offset,
        ap=list(base.ap[:1]) + [[SL, 3], [1, LOD], [1, T]],
    )


def _rep3(plane_ap):
    """Broadcast a [128, 448] plane to [128, 3, 448] with a 0-stride dim."""
    return AP(
        tensor=plane_ap.tensor,
        offset=plane_ap.offset,
        ap=list(plane_ap.ap[:1]) + [[0, 3], [1, PL]],
    )


def _strip_dead_self_waits(nc):
    """Remove same-engine sem waits already satisfied by program order.

    Tile's sem assignment emits conservative same-engine waits when its
    scheduler reorders a stream; walrus caps waits at 1 per instruction, so
    drop any wait on semaphore S whose value is <= the inc-count of S from
    instructions earlier in the stream (provably satisfied at issue time).
    """
    eng_sem = {
        mybir.EngineType.Activation: "Activation_44",
        mybir.EngineType.PE: "PE_44",
        mybir.EngineType.DVE: "DVE_44",
        mybir.EngineType.Pool: "Pool_44",
        mybir.EngineType.SP: "SP_44",
    }
    inc_count = {}
    for inst in nc.all_instructions():
        si = getattr(inst, "sync_info", None)
        if si is None:
            continue
        own = eng_sem.get(getattr(inst, "engine", None))
        if si.on_wait and own is not None:
            keep = []
            for w in si.on_wait:
                # only self-engine waits are provably ordered by the stream
                if (
                    w.ant_name == own
                    and w.wait_mode == "sem-ge-imm"
                    and inc_count.get(own, 0) >= (w.wait_value or 0)
                ):
                    continue
                keep.append(w)
            if len(keep) != len(si.on_wait):
                si.on_wait = keep
        for u in si.on_update:
            if u.update_mode == "sem-inc":
                inc_count[u.ant_name] = inc_count.get(u.ant_name, 0) + (
                    u.update_value or 0
                )


def _split_multi_waits(nc, cap=1):
    """Walrus caps sync-waits per instruction; spread extras over inserted
    drains on the same engine immediately before the offender."""
    for blk in nc.main_func.blocks:
        insts = blk.instructions
        i = 0
        while i < len(insts):
            inst = insts[i]
            si = getattr(inst, "sync_info", None)
            if si is not None and si.on_wait and len(si.on_wait) > cap:
                waits = list(si.on_wait)
                si.on_wait = waits[-cap:]
                extras = waits[:-cap]
                for j, w in enumerate(extras[::-1]):
                    d = mybir.InstDrain(
                        name=f"{inst.name}_wsplit{j}",
                        engine=inst.engine,
                        ins=[],
                        outs=[],
                        sync_info=mybir.SyncInfo(on_wait=[w], on_update=[]),
                    )
                    nc.register_instruction(d)
                    insts.insert(i, d)
                i += len(extras)
            i += 1


def build_bass():
    nc = bass.Bass()

    pm_d = nc.dram_tensor("pm", [R, LSD], F32, kind="ExternalInput")
    cov_d = nc.dram_tensor("cov", [R, 3 * LOD], F32, kind="ExternalInput")
    cbf_d = nc.dram_tensor("cbf", [P, CB_N], BF16, kind="ExternalInput")
    cf32_d = nc.dram_tensor("cf32", [P, CF_N], F32, kind="ExternalInput")
    out_d = nc.dram_tensor("out", [R, 5 * LOD], F32, kind="ExternalOutput")

    with TileContext(nc) as tc, ExitStack() as ctx:
        const = ctx.enter_context(tc.tile_pool(name="const", bufs=1))
        work = ctx.enter_context(tc.tile_pool(name="work", bufs=3))
        ps_sm = ctx.enter_context(tc.tile_pool(name="ps_sm", bufs=3, space="PSUM"))
        ps_d = ctx.enter_context(tc.tile_pool(name="ps_d", bufs=1, space="PSUM"))
        ps_w = ctx.enter_context(tc.tile_pool(name="ps_w", bufs=1, space="PSUM"))

        # ---- stage everything with one DMA per DRAM tensor ----
        pm_sb = const.tile([P, NT * LSD], F32)
        cov_sb = const.tile([P, NT * 3 * LOD], F32)
        for t in range(NT):
            nc.sync.dma_start(
                pm_sb[:, t * LSD : (t + 1) * LSD], pm_d[t * P : (t + 1) * P, :]
            )
            nc.sync.dma_start(
                cov_sb[:, t * 3 * LOD : (t + 1) * 3 * LOD],
                cov_d[t * P : (t + 1) * P, :],
            )
        cbf_sb = const.tile([P, CB_N], BF16)
        nc.sync.dma_start(cbf_sb[:], cbf_d[:])
        cf32_sb = const.tile([P, CF_N], F32)
        nc.sync.dma_start(cf32_sb[:], cf32_d[:])

        e_sb = cbf_sb[0:K, CB_E : CB_E + 4 * PL]
        w1_sb = cbf_sb[:, CB_W1 : CB_W1 + H]
        w2_sb = cbf_sb[:, CB_W2 : CB_W2 + K]
        idn_sb = cf32_sb[:, CF_IDN : CF_IDN + P]
        pcb_sb = cf32_sb[:, CF_PCB : CF_PCB + 3 * LOD]
        b1_sb = cf32_sb[:, CF_B1 : CF_B1 + 1]
        b2_sb = cf32_sb[0:K, CF_B2 : CF_B2 + 1]

        outb = const.tile([P, NT * 5 * LOD], F32)

        # ---- absorbers: put each DMA queue on each engine's clock ----
        absb = const.tile([1, 8], BF16)
        absf = const.tile([1, 8], F32)
        nc.vector.tensor_copy(absf[0:1, 0:1], pm_sb[0:1, 0:1])
        nc.vector.tensor_copy(absf[0:1, 1:2], cov_sb[0:1, 0:1])
        nc.vector.tensor_copy(absb[0:1, 0:1], cbf_sb[0:1, 0:1])
        nc.vector.tensor_copy(absf[0:1, 2:3], cf32_sb[0:1, 0:1])
        nc.scalar.copy(absf[0:1, 3:4], cf32_sb[0:1, 0:1])
        warm_ps = ps_w.tile([1, 8], F32, tag="warm")
        nc.tensor.matmul(warm_ps[0:1, 0:1], cbf_sb[0:1, 0:1], cbf_sb[0:1, 0:1])
        nc.tensor.matmul(warm_ps[0:1, 1:2], cf32_sb[0:1, 0:1], cf32_sb[0:1, 0:1])

        # basepc for all four tiles in one op: cov_sb + pcb broadcast over tiles
        basepc4 = const.tile([P, NT * 3 * LOD], F32)
        nc.vector.tensor_tensor(
            basepc4[:].rearrange("p (t c) -> p t c", t=NT),
            cov_sb[:].rearrange("p (t c) -> p t c", t=NT),
            AP(
                tensor=pcb_sb.tensor,
                offset=pcb_sb.offset,
                ap=list(pcb_sb.ap[:1]) + [[0, NT], [1, 3 * LOD]],
            ),
            OP.add,
        )

        for it in range(NT):
            pm_f = pm_sb[:, it * LSD : (it + 1) * LSD]
            covcat = cov_sb[:, it * 3 * LOD : (it + 1) * 3 * LOD]

            # ---- MLP + softmax (b-partition layout at the end) ----
            pmT_ps = ps_sm.tile([P, P], F32, tag="sm")
            nc.tensor.transpose(pmT_ps[:], pm_f, idn_sb)
            pmT_bf = work.tile([P, P], BF16, tag="pmT")
            nc.scalar.copy(pmT_bf[:], pmT_ps[:])

            h_ps = ps_sm.tile([P, P], F32, tag="sm")
            nc.tensor.matmul(h_ps[:], w1_sb, pmT_bf[:])  # [H, b]
            h_bf = work.tile([P, P], BF16, tag="h")
            nc.scalar.activation(h_bf[:], h_ps[:], AF.Tanh, bias=b1_sb)

            lg_ps = ps_sm.tile([K, P], F32, tag="sm")
            nc.tensor.matmul(lg_ps[:], w2_sb, h_bf[:])  # [K, b]
            lg_sb = work.tile([K, P], F32, tag="lg")
            nc.scalar.activation(lg_sb[:], lg_ps[:], AF.Identity, bias=b2_sb)

            lgT_ps = ps_sm.tile([P, K], F32, tag="sm")
            nc.tensor.transpose(lgT_ps[:], lg_sb[:], idn_sb[0:K, 0:K])
            e_t = work.tile([P, K], F32, tag="esb")
            ssum = work.tile([P, 1], F32, tag="ssum")
            nc.scalar.activation(e_t[:], lgT_ps[:], AF.Exp, accum_out=ssum[:])

            r_t = work.tile([P, 1], F32, tag="rt")
            nc.vector.reciprocal(r_t[:], ssum[:])
            coef = work.tile([P, K], F32, tag="coef")
            nc.scalar.mul(coef[:], e_t[:], r_t[:, 0:1])

            coefT_ps = ps_sm.tile([K, P], F32, tag="sm")
            nc.tensor.transpose(coefT_ps[:], coef[:], idn_sb)
            coefT = work.tile([K, P], BF16, tag="coefT")
            nc.scalar.copy(coefT[:], coefT_ps[:])

            # ---- D planes: [b, m, i, t] via PE, evac to bf16 ----
            d_ps = ps_d.tile([P, 2048], F32, tag="D")
            for m in range(4):
                nc.tensor.matmul(
                    d_ps[:, 512 * m : 512 * m + PL],
                    coefT[:],
                    e_sb[:, PL * m : PL * (m + 1)],
                )
            d_bf = work.tile([P, 4 * PL], BF16, tag="D")
            nc.scalar.copy(
                d_bf[:].rearrange("p (m x) -> p m x", m=4),
                d_ps[:].rearrange("p (m x) -> p m x", m=4)[:, :, 0:PL],
            )

            # ---- x6p: zero-padded bf16 slots (mu, cu, cs, ml, cs, cl) ----
            # built on ACT; pads only need zeroing while the pool bufs are fresh
            x6p = work.tile([P, 6 * SL], BF16, tag="x6p")
            nc.scalar.memzero(x6p[:])

            def ap2(base, off, step):  # [128, 2, 64] strided pair view
                b = base[:, off : off + 1]
                return AP(
                    tensor=b.tensor,
                    offset=b.offset,
                    ap=list(b.ap[:1]) + [[step, 2], [1, LOD]],
                )

            # slots (mu@0, ml@3): from pm columns (0, 64); dst stride 3*SL
            nc.scalar.copy(ap2(x6p, BW, 3 * SL), ap2(pm_sb, it * LSD, LOD))
            # slots (cu@1, cs@2): from cov columns (0, 128); dst stride SL
            nc.scalar.copy(ap2(x6p, SL + BW, SL), ap2(cov_sb, it * 3 * LOD, 2 * LOD))
            # slots (cs@4, cl@5): from cov columns (128, 64); dst stride SL
            nc.scalar.copy(
                ap2(x6p, 4 * SL + BW, SL),
                ap2(cov_sb, it * 3 * LOD + 2 * LOD, -LOD),
            )

            # ---- banded TT pipeline (bf16, DVE) ----
            d1r3 = _rep3(d_bf[:, 0:PL])
            d2r3 = _rep3(d_bf[:, PL : 2 * PL])
            d3r3 = _rep3(d_bf[:, 2 * PL : 3 * PL])
            d4r3 = _rep3(d_bf[:, 3 * PL : 4 * PL])
            sA = _sread(x6p, 0)
            sB = _sread(x6p, 3)

            tmpA = work.tile([P, 3 * PL], BF16, tag="tmpA")
            tmpB = work.tile([P, 3 * PL], BF16, tag="tmpB")
            upp = work.tile([P, 6 * PL], BF16, tag="upp")
            # chunk layout in upp: U1@0, P1@448, P2@896, U2@1344, P3@1792, P4@2240
            nc.vector.tensor_tensor(
                tmpA[:].rearrange("p (s x) -> p s x", s=3), d1r3, sA, OP.mult
            )
            nc.vector.tensor_tensor(
                tmpB[:].rearrange("p (s x) -> p s x", s=3), d2r3, sB, OP.mult
            )
            nc.vector.tensor_add(upp[:, 0 : 3 * PL], tmpA[:], tmpB[:])
            nc.vector.tensor_tensor(
                tmpA[:].rearrange("p (s x) -> p s x", s=3), d3r3, sA, OP.mult
            )
            nc.vector.tensor_tensor(
                tmpB[:].rearrange("p (s x) -> p s x", s=3), d4r3, sB, OP.mult
            )
            nc.vector.tensor_add(upp[:, 3 * PL : 6 * PL], tmpA[:], tmpB[:])

            def _pair(base_tile, off, step, inner):
                b = base_tile[:, off : off + 1]
                return AP(
                    tensor=b.tensor,
                    offset=b.offset,
                    ap=list(b.ap[:1]) + [[step, 2], [1, inner]],
                )

            # vab = (D1P1, D2P2, D3P3, D4P4 | D3P1, D4P2)
            vab = work.tile([P, 6 * PL], BF16, tag="vab")
            nc.vector.tensor_tensor(
                vab[:, 0 : 4 * PL].rearrange("p (u x) -> p u x", u=2),
                d_bf[:].rearrange("p (u x) -> p u x", u=2),
                _pair(upp, PL, 3 * PL, 2 * PL),
                OP.mult,
            )
            nc.vector.tensor_mul(
                vab[:, 4 * PL : 6 * PL], d_bf[:, 2 * PL : 4 * PL], upp[:, PL : 3 * PL]
            )
            # token: pulls DVE's clock onto ACT so next tile's PSUM evacs
            # don't need explicit DVE waits (per-instruction wait cap is 1)
            tok = work.tile([P, 1], BF16, tag="tok")
            nc.scalar.copy(tok[:], vab[:, 0:1])

            # covq3 = (Q1+Q2, Q3+Q4, R1+R2) in one add
            covq3 = work.tile([P, 3 * PL], BF16, tag="covq3")
            nc.vector.tensor_tensor(
                covq3[:].rearrange("p (u x) -> p u x", u=3),
                AP(
                    tensor=vab[:].tensor,
                    offset=vab[:].offset,
                    ap=list(vab[:].ap[:1]) + [[2 * PL, 3], [1, PL]],
                ),
                AP(
                    tensor=vab[:, PL : PL + 1].tensor,
                    offset=vab[:, PL : PL + 1].offset,
                    ap=list(vab[:].ap[:1]) + [[2 * PL, 3], [1, PL]],
                ),
                OP.add,
            )

            ured = work.tile([P, 2 * LOD], F32, tag="ured")
            nc.vector.reduce_sum(
                ured[:].rearrange("p (u i) -> p u i", u=2),
                AP(
                    tensor=upp[:].tensor,
                    offset=upp[:].offset,
                    ap=list(upp[:].ap[:1]) + [[3 * PL, 2], [T, LOD], [1, T]],
                ),
                axis=AX.X,
            )
            covqall = work.tile([P, 3 * LOD], F32, tag="covqall")
            nc.vector.reduce_sum(
                covqall[:].rearrange("p (u i) -> p u i", u=3),
                covq3[:].rearrange("p (u i t) -> p u i t", u=3, t=T),
                axis=AX.X,
            )

            # ---- assembly into the staged output buffer ----
            o0 = it * 5 * LOD
            basepc = basepc4[:, it * 3 * LOD : (it + 1) * 3 * LOD]

            nc.vector.tensor_add(outb[:, o0 : o0 + 128], ured[:], pm_f)

            def pslice(off):  # t=3 slice of upp chunk at column off
                return upp[:, off : off + PL].rearrange("p (i t) -> p i t", t=T)[
                    :, :, 3
                ]

            tmc = work.tile([P, 3 * LOD], F32, tag="tmc")
            # ncu/ncl fused: in0 = (P1_3, P4_3) stride 4*PL
            nc.vector.scalar_tensor_tensor(
                tmc[:, 0:128].rearrange("p (u i) -> p u i", u=2),
                AP(
                    tensor=upp[:, PL + 3 : PL + 4].tensor,
                    offset=upp[:, PL + 3 : PL + 4].offset,
                    ap=list(upp[:].ap[:1]) + [[4 * PL, 2], [T, LOD]],
                ),
                2.0,
                covqall[:, 0:128].rearrange("p (u i) -> p u i", u=2),
                OP.mult,
                OP.add,
            )
            nc.vector.tensor_add(tmc[:, 128:192], pslice(2 * PL), pslice(4 * PL))
            nc.vector.tensor_add(
                tmc[:, 128:192], tmc[:, 128:192], covqall[:, 128:192]
            )
            nc.vector.tensor_add(
                outb[:, o0 + 128 : o0 + 320], tmc[:], basepc
            )

        for t in range(NT):
            nc.sync.dma_start(
                out_d[t * P : (t + 1) * P, :],
                outb[:, t * 5 * LOD : (t + 1) * 5 * LOD],
            )

    _split_multi_waits(nc)
    return nc


_NC_CACHE = None


def _get_nc():
    global _NC_CACHE
    if _NC_CACHE is None:
        _NC_CACHE = build_bass()
    return _NC_CACHE


def _prep_aux(inputs):
    bsm = [inputs["basis11"], inputs["basis12"], inputs["basis21"], inputs["basis22"]]
    E = np.zeros((K, 4, LOD, T), np.float32)
    for m in range(4):
        for t in range(T):
            off = t - BW
            lo, hi = max(0, -off), min(LOD, LOD - off)
            E[:, m, lo:hi, t] = bsm[m][:, np.arange(lo, hi), np.arange(lo, hi) + off]

    cbf = np.zeros((P, CB_N), ml_dtypes.bfloat16)
    cbf[0:K, CB_E : CB_E + 4 * PL] = E.reshape(K, 4 * PL).astype(ml_dtypes.bfloat16)
    cbf[:, CB_W1 : CB_W1 + H] = inputs["coeff_w1"].T.astype(ml_dtypes.bfloat16)
    cbf[:, CB_W2 : CB_W2 + K] = inputs["coeff_w2"].T.astype(ml_dtypes.bfloat16)

    cf32 = np.zeros((P, CF_N), np.float32)
    cf32[:, CF_IDN : CF_IDN + P] = np.eye(P, dtype=np.float32)
    lpn = inputs["log_process_noise"].astype(np.float32)
    pc = np.where(lpn < 0, np.exp(lpn), lpn + 1.0)[0]
    pcb_row = np.concatenate([pc[:LOD], pc[LOD:], np.zeros(LOD, np.float32)])
    cf32[:, CF_PCB : CF_PCB + 3 * LOD] = pcb_row
    cf32[:, CF_B1] = inputs["coeff_b1"].astype(np.float32)
    cf32[0:K, CF_B2] = inputs["coeff_b2"].astype(np.float32)
    return dict(cbf=cbf, cf32=cf32)


def kernel(**inputs):
    return _run(inputs, trace=False)[0]


def _run(inputs, trace=False, tmpdir=None):
    inputs = {k: np.asarray(v) for k, v in inputs.items()}
    aux = _prep_aux(inputs)
    nc = _get_nc()

    cov_full = np.concatenate(
        [inputs["post_cov_u"], inputs["post_cov_l"], inputs["post_cov_s"]], axis=1
    ).astype(np.float32)

    in_maps = []
    for c in range(NCORES):
        sl = slice(c * R, (c + 1) * R)
        m = dict(aux)
        m["pm"] = np.ascontiguousarray(inputs["post_mean"][sl]).astype(np.float32)
        m["cov"] = np.ascontiguousarray(cov_full[sl])
        in_maps.append(m)

    res = run_bass_kernel_spmd(
        nc, in_maps, list(range(NCORES)), trace=trace, tmpdir=tmpdir
    )
    outs = [np.asarray(res.results[c]["out"]) for c in range(NCORES)]
    return np.concatenate(outs, axis=0).astype(np.float32), res
